# revision 61
# baseline (speedup 1.0000x reference)
"""Self-contained Trainium2 Bass kernel for nn_AttentionBlock
(B=2, N=2048, D=512, H=8, MLP 2x).

kernel(**inputs) takes the FULL unsharded inputs (as produced by
setup_inputs) and returns the FULL (2, 2048, 512) output.

Sharding: 2-way data-parallel over batch x 4-way parallel over query-token
slices (8 cores, no collectives).  Each core computes K/V for its whole
batch and attention + MLP for its 512-token slice; the host stitches.

v2: fp8(e4m3) DoubleRow matmuls for all deep GEMMs (projections, AV,
MLP) and zero-padded DoubleRow for the 64-deep attention scores;
weights/activations quantized host-side; PE-based row broadcasts;
rstd via exp(-0.5*ln(var)) keeps the Act engine on one function table.
"""

from contextlib import ExitStack

import numpy as np
import ml_dtypes

import concourse.bass as bass
import concourse.mybir as mybir
import concourse.tile as tile

_WSPLIT_UID = [0]


def _finalize(nc, max_waits=1):
    """Split multi-sem-wait instructions onto single-wait NoOp carriers
    (the walrus build in this container accepts one wait per instruction)."""
    for f in nc.m.functions:
        for bb in f.blocks:
            insts = bb.instructions
            out = []
            changed = False
            for inst in insts:
                si = inst.sync_info
                waits = list(si.on_wait) if (si and si.on_wait) else []
                if len(waits) > max_waits:
                    changed = True
                    for w in waits[:-max_waits]:
                        _WSPLIT_UID[0] += 1
                        nop = mybir.InstNoOp(
                            name=f"I-wsplit-{_WSPLIT_UID[0]}",
                            ins=[], outs=[], engine=inst.engine,
                        )
                        nop.sync_info = mybir.SyncInfo(on_wait=[w],
                                                       on_update=[])
                        out.append(nop)
                    si.on_wait = waits[-max_waits:]
                out.append(inst)
            if changed:
                bb.instructions = out
    return nc

BF16 = mybir.dt.bfloat16
F32 = mybir.dt.float32
F32R = mybir.dt.float32r
FP8 = mybir.dt.float8e4
AF = mybir.ActivationFunctionType
OP = mybir.AluOpType
DR = mybir.MatmulPerfMode.DoubleRow

P = 128
B, N, D, H = 2, 2048, 512, 8
HD = D // H          # 64
TC = 512             # tokens per core
DM = 2 * D           # 1024 mlp hidden
KC = D // P          # 4 chunks of the 512 feature dim
NT = N // 512        # 4 tiles of 512 over the 2048 kv tokens
JC = N // P          # 16 token chunks of 128 over kv tokens
MC1 = DM // P        # 8 chunks of mlp hidden
VW = HD + 1          # V row width (ones column for the softmax denom)
MB = N + P           # per head-pair block width in KT8 (incl. 128 pad)

BQ, NSQ, NSK, NSV = 0, 1, 2, 3  # crow rows


def dram_bcast_src(dram_ap, nparts):
    """AP re-reading a [1, n] DRAM row on `nparts` partitions (DMA src)."""
    return bass.AP(
        tensor=dram_ap.tensor,
        offset=dram_ap.offset,
        ap=[[0, nparts]] + [list(x) for x in dram_ap.ap[1:]],
    )


def build_nc(st_bufs=16, do_finalize=True):
    nc = bass.Bass()
    y8T = nc.dram_tensor("y8T", [D, N], FP8, kind="ExternalInput")
    yoT = nc.dram_tensor("yoT", [D, TC], F32, kind="ExternalInput")
    Wq8 = nc.dram_tensor("Wq8", [D, D], FP8, kind="ExternalInput")
    Wk8 = nc.dram_tensor("Wk8", [D, D], FP8, kind="ExternalInput")
    Wv8 = nc.dram_tensor("Wv8", [D, D], FP8, kind="ExternalInput")
    Wo8 = nc.dram_tensor("Wo8", [D, D], FP8, kind="ExternalInput")
    W18 = nc.dram_tensor("W18", [D, DM], BF16, kind="ExternalInput")
    W28 = nc.dram_tensor("W28", [DM, D], BF16, kind="ExternalInput")
    crow = nc.dram_tensor("crow", [4, D], BF16, kind="ExternalInput")
    ccol = nc.dram_tensor("ccol", [P, 16], F32, kind="ExternalInput")
    outT = nc.dram_tensor("outT", [D, TC], F32, kind="ExternalOutput")

    y8r = y8T.rearrange("(o p) t -> p o t", p=P)
    yor = yoT.rearrange("(o p) t -> p o t", p=P)
    wq8r = Wq8.rearrange("(o p) n -> p o n", p=P)
    wk8r = Wk8.rearrange("(o p) n -> p o n", p=P)
    wv8r = Wv8.rearrange("(o p) n -> p o n", p=P)
    wo8r = Wo8.rearrange("(o p) n -> p o n", p=P)
    w18r = W18.rearrange("(o p) n -> p o n", p=P)
    w28r = W28.rearrange("(o p) n -> p o n", p=P)

    with tile.TileContext(nc, pool_alloc_mode="queue") as tc:
        with (
            tc.tile_pool(name="const", bufs=1) as const,
            tc.tile_pool(name="xp", bufs=1) as xp,
            tc.tile_pool(name="rows", bufs=1) as rows,
            tc.tile_pool(name="rtmp", bufs=4) as rtmp,
            tc.tile_pool(name="dpool", bufs=2, space="DRAM") as dpool,
            tc.tile_pool(name="kqv", bufs=1) as kqv,
            tc.tile_pool(name="rt8p", bufs=1) as rt8p,
            tc.tile_pool(name="y2p", bufs=1) as y2p,
        ):
            # ---- constants (bf16: matmuls may not mix f32r with other
            # dtypes on hardware, so rows/consts are uniformly bf16) ----
            ident = const.tile([1, 1], BF16)
            nc.vector.memset(ident[:], 1.0)
            onec_f = const.tile([P, 1], F32, name="onec_f")
            nc.vector.memset(onec_f[:], 1.0)
            onec_r = const.tile([P, 1], F32R, name="onec_r")
            nc.gpsimd.tensor_copy(out=onec_r[:], in_=onec_f[:])
            onec_b = const.tile([P, 1], BF16, name="onec_b")
            nc.vector.memset(onec_b[:], 1.0)
            ones_row = const.tile([1, P], BF16, name="ones_row")
            nc.vector.memset(ones_row[:], 1.0)
            oned_row = const.tile([1, P], BF16, name="oned_row")
            nc.vector.memset(oned_row[:], 1.0 / 512.0)
            ccol_sb = const.tile([P, 16], F32)
            crow_sb = const.tile([1, 4, D], BF16, name="crow_sb")
            crow_r = crow_sb
            # Act table warm-up: first real Act op shouldn't pay the
            # 1283ns table load on the critical path.
            warm = const.tile([1, 1], F32, name="warm")
            nc.scalar.activation(out=warm[:], in_=ident[:], func=AF.Ln)

            # ---- big SBUF tensors ----
            y8 = xp.tile([P, KC, N], FP8, name="y8")
            sq8 = xp.tile([P, KC, N], FP8, name="sq8")
            xo = xp.tile([P, KC, TC], F32, name="xo")
            Wq_s = xp.tile([P, KC, D], FP8, name="Wq_s")
            Wk_s = xp.tile([P, KC, D], FP8, name="Wk_s")
            Wv_s = xp.tile([P, KC, D], FP8, name="Wv_s")
            Wo_s = xp.tile([P, KC, D], FP8, name="Wo_s")
            W1_s = xp.tile([P, KC, DM], BF16, name="W1_s")
            W2_s = xp.tile([P, MC1, D], BF16, name="W2_s")
            KT8 = kqv.tile([P, KC * MB], FP8, name="KT8")
            QT8 = kqv.tile([P, KC, 2, TC], FP8, name="QT8")
            V8 = kqv.tile([P, JC, H, VW], FP8, name="V8")
            RT8 = rt8p.tile([P, KC, TC], FP8, name="RT8")
            y2T = y2p.tile([P, KC, TC], F32R, name="y2T")

            # input DMAs: y8 first (stats critical path), split across
            # SP/Pool queues; everything else behind it.
            crow_v = crow.rearrange("(o r) d -> o r d", o=1)
            for i in range(NT):
                ts = slice(i * 512, (i + 1) * 512)
                eng = nc.sync if i % 2 == 0 else nc.gpsimd
                eng.dma_start(out=y8[:, :, ts], in_=y8r[:, :, ts])
            nc.sync.dma_start(out=Wq_s[:], in_=wq8r[:])
            nc.sync.dma_start(out=crow_sb[:], in_=crow_v[:])
            nc.sync.dma_start(out=Wk_s[:], in_=wk8r[:])
            nc.sync.dma_start(out=Wv_s[:], in_=wv8r[:])
            nc.sync.dma_start(out=ccol_sb[:], in_=ccol[:])
            for i in range(2):
                cs = slice(i * 256, (i + 1) * 256)
                nc.sync.dma_start(out=xo[:, :, cs], in_=yor[:, :, cs])
            # phase-C weights up front so their DMAs don't wait on the
            # SBUF ring recycling mid-attention
            nc.sync.dma_start(out=Wo_s[:], in_=wo8r[:])
            nc.sync.dma_start(out=W1_s[:], in_=w18r[:])
            nc.sync.dma_start(out=W2_s[:], in_=w28r[:])

            # rows (bf16 so they can pair with bf16 consts in matmuls)
            S_row = rows.tile([1, N], BF16, name="S_row")
            rstd_row = rows.tile([1, N], BF16, name="rstd_row")
            sd_row = rows.tile([1, TC], BF16, name="sd_row")
            rstd_tok = rows.tile([P, JC], F32, name="rstd_tok")
            arep_sb = rows.tile([P, NT, 512], BF16, name="arep_sb")
            Sr = S_row

            a_stack = ExitStack()
            pstat = a_stack.enter_context(
                tc.tile_pool(name="pstat", bufs=2, space="PSUM"))
            parep = a_stack.enter_context(
                tc.tile_pool(name="parep", bufs=2, space="PSUM"))
            ppt = a_stack.enter_context(
                tc.tile_pool(name="ppt", bufs=2, space="PSUM"))

            # ================= LN1 stats (all 4 tiles) =================
            # sq8 is produced in quarter-chunks split across Pool/DVE so
            # the DVE row chain (S copy, var) is never stuck behind a big
            # squaring op; ps accumulates per quarter.
            for nt in range(NT):
                ts = slice(nt * 512, nt * 512 + 512)
                pm = pstat.tile([1, 512], F32, name="pm", tag="pm")
                for q in range(KC):
                    nc.tensor.matmul(pm[:], onec_b[:], y8[:, q, ts],
                                     start=(q == 0), stop=(q == KC - 1))
                ps = pstat.tile([1, 512], F32, name="ps", tag="pm")
                for q in range(KC):
                    with nc.allow_low_precision(reason="fp8 x^2"):
                        nc.gpsimd.tensor_tensor(out=sq8[:, q, ts],
                                                in0=y8[:, q, ts],
                                                in1=y8[:, q, ts], op=OP.mult)
                    nc.tensor.matmul(ps[:], onec_b[:], sq8[:, q, ts],
                                     start=(q == 0), stop=(q == KC - 1))
                with nc.allow_low_precision(reason="bf16 rows"):
                    nc.vector.tensor_copy(out=S_row[:, ts], in_=pm[:])
                t_row = rtmp.tile([1, 512], F32, name="t_row", tag="t_row")
                nc.scalar.activation(out=t_row[:], in_=pm[:], func=AF.Square)
                var_row = rtmp.tile([1, 512], F32, name="var_row",
                                    tag="var_row")
                nc.vector.scalar_tensor_tensor(
                    out=var_row[:], in0=t_row[:], scalar=-1.0 / 512.0,
                    in1=ps[:], op0=OP.mult, op1=OP.add)
                ln_row = rtmp.tile([1, 512], F32, name="ln_row", tag="ln_row")
                nc.scalar.activation(out=ln_row[:], in_=var_row[:],
                                     func=AF.Ln, scale=1.0 / 512.0)
                with nc.allow_low_precision(reason="bf16 rows"):
                    nc.scalar.activation(out=rstd_row[:, ts], in_=ln_row[:],
                                         func=AF.Exp, scale=-0.5)
                    if nt == 0:
                        nc.scalar.activation(out=sd_row[:], in_=ln_row[:],
                                             func=AF.Exp, scale=0.5)
                # replicate rstd across partitions via PE, stash as bf16
                pa = parep.tile([P, 512], F32, name="pa", tag="pa")
                nc.tensor.matmul(pa[:], ones_row[:],
                                 rstd_row[:, ts],
                                 start=True, stop=True)
                with nc.allow_low_precision(reason="bf16 rstd replica"):
                    nc.vector.tensor_copy(out=arep_sb[:, nt, :], in_=pa[:])
                # per-token rstd columns (for the V eviction scale)
                for jc in range(nt * 4, nt * 4 + 4):
                    pt = ppt.tile([P, 1], BF16, name="pt", tag="pt")
                    nc.tensor.transpose(
                        pt[:], rstd_row[:, jc * P:(jc + 1) * P], ident[:])
                    with nc.allow_low_precision(reason="bf16 rows"):
                        nc.vector.tensor_copy(out=rstd_tok[:, jc:jc + 1],
                                              in_=pt[:])
            a_stack.close()

            # zero pads (needed before scores/AV, issued off the critical
            # startup path): QT8 z=1 blocks, KT8 per-m pads, V ones column
            with nc.allow_low_precision(reason="fp8 zeros"):
                nc.gpsimd.memset(QT8[:, :, 1, :], 0.0)
                nc.gpsimd.memset(
                    KT8[:].rearrange("p (o t) -> p o t", o=KC)[:, :, N:MB],
                    0.0)
                nc.gpsimd.memset(V8[:, :, :, HD:VW], 8.0)

            # ================= Q projection (tile 0 tokens) =================
            with tc.tile_pool(name="pkv", bufs=2, space="PSUM") as pkv:
                for m in range(KC):
                    ms = slice(m * P, m * P + P)
                    pq = pkv.tile([P, 512], F32, name="pq", tag="pk")
                    for t in range(2):
                        nc.tensor.matmul(pq[:], Wq_s[:, 2 * t:2 * t + 2, ms],
                                         y8[:, 2 * t:2 * t + 2, 0:TC],
                                         start=(t == 0), stop=False,
                                         perf_mode=DR)
                    nc.tensor.matmul(pq[:], crow_r[:, NSQ, ms], Sr[:, 0:TC],
                                     start=False, stop=False)
                    nc.tensor.matmul(pq[:], crow_r[:, BQ, ms],
                                     sd_row[:],
                                     start=False, stop=True)
                    with nc.allow_low_precision(reason="fp8 evict"):
                        nc.vector.tensor_tensor(out=QT8[:, m, 0, :],
                                                in0=pq[:],
                                                in1=arep_sb[:, 0, :],
                                                op=OP.mult)

                # ========== K proj + scores (m-major), V, AV ==========
                with (
                    tc.tile_pool(name="pss", bufs=2, space="PSUM") as pss,
                    tc.tile_pool(name="stp", bufs=st_bufs) as stp,
                    tc.tile_pool(name="prp", bufs=2, space="PSUM") as prp,
                    tc.tile_pool(name="rsp", bufs=4) as rsp,
                ):
                    KTv = KT8[:].rearrange("p (o t) -> p o t", o=KC)

                    def kproj(m):
                        ms = slice(m * P, m * P + P)
                        for nt in range(NT):
                            ts = slice(nt * 512, nt * 512 + 512)
                            pk = pkv.tile([P, 512], F32, name="pk", tag="pk")
                            for t in range(2):
                                nc.tensor.matmul(
                                    pk[:], Wk_s[:, 2 * t:2 * t + 2, ms],
                                    y8[:, 2 * t:2 * t + 2, ts],
                                    start=(t == 0), stop=False, perf_mode=DR)
                            nc.tensor.matmul(pk[:], crow_r[:, NSK, ms],
                                             Sr[:, ts],
                                             start=False, stop=True)
                            with nc.allow_low_precision(reason="fp8 evict"):
                                nc.vector.tensor_tensor(out=KTv[:, m, ts],
                                                  in0=pk[:],
                                                  in1=arep_sb[:, nt, :],
                                                  op=OP.mult)

                    def scores(m):
                        for jp in range(JC // 2):
                            for r in range(2):
                                hs = slice(r * HD, r * HD + HD)
                                psc = pss.tile([P, 1024], F32, name="psc",
                                               tag="psc")
                                for half in range(2):
                                    jc = jp * 2 + half
                                    lhs = KT8[hs, m * MB + jc * P:
                                              m * MB + jc * P + 2 * P]
                                    lhs = lhs.rearrange("p (z t) -> p z t",
                                                        z=2)
                                    nc.tensor.matmul(
                                        psc[:, half * 512:half * 512 + 512],
                                        lhs, QT8[hs, m, :, :],
                                        start=True, stop=True, perf_mode=DR)
                                st = stp.tile([P, 1024], FP8, name="st",
                                              tag=f"st{r}")
                                with nc.allow_low_precision(reason="fp8 st"):
                                    nc.scalar.activation(
                                        out=st[:], in_=psc[:], func=AF.Exp,
                                        scale=1.0 / 512.0)
                                st_all[2 * m + r].append(st)

                    def vproj():
                        for jc in range(JC):
                            js = slice(jc * P, jc * P + P)
                            pv = pkv.tile([P, 512], F32, name="pv", tag="pk")
                            for t in range(2):
                                nc.tensor.matmul(
                                    pv[:], y8[:, 2 * t:2 * t + 2, js],
                                    Wv_s[:, 2 * t:2 * t + 2, :],
                                    start=(t == 0), stop=False, perf_mode=DR)
                            nc.tensor.matmul(pv[:], Sr[:, js],
                                             crow_r[:, NSV, :],
                                             start=False, stop=True)
                            with nc.allow_low_precision(reason="fp8 evict"):
                                nc.vector.tensor_tensor(
                                    out=V8[:, jc, :, 0:HD],
                                    in0=pv.rearrange("p (h c) -> p h c", h=H),
                                    in1=rstd_tok[:, jc:jc + 1].to_broadcast(
                                        (P, H, HD)),
                                    op=OP.mult)

                    def av(h):
                        m, r = h // 2, h % 2
                        hs = slice(r * HD, r * HD + HD)
                        pr = prp.tile([VW, 512], F32, name="pr", tag="pr")
                        for jp in range(JC // 2):
                            nc.tensor.matmul(
                                pr[:], V8[:, 2 * jp:2 * jp + 2, h, :],
                                st_all[h][jp][:].rearrange(
                                    "p (z t) -> p z t", z=2),
                                start=(jp == 0), stop=(jp == JC // 2 - 1),
                                perf_mode=DR)
                        rs_row = rsp.tile([1, TC], BF16, name="rs_row",
                                          tag="rs")
                        with nc.allow_low_precision(reason="recip"):
                            nc.vector.reciprocal(out=rs_row[:],
                                                 in_=pr[HD:HD + 1, :])
                        rs_dr = dpool.tile([1, TC], BF16, name="rs_dr",
                                           tag="rsd")
                        eng = nc.sync if h % 2 == 0 else nc.gpsimd
                        eng.dma_start(out=rs_dr[:], in_=rs_row[:])
                        rrep = rsp.tile([HD, TC], BF16, name="rrep",
                                        tag="rrep")
                        eng.dma_start(out=rrep[:],
                                      in_=dram_bcast_src(rs_dr[:], HD))
                        with nc.allow_low_precision(reason="fp8 evict"):
                            nc.vector.tensor_tensor(out=RT8[hs, m, :],
                                                    in0=pr[0:HD, :],
                                                    in1=rrep[:], op=OP.mult)

                    st_all = {h: [] for h in range(H)}
                    kproj(0)
                    scores(0)
                    vproj()
                    kproj(1)
                    scores(1)
                    av(0)
                    av(1)
                    kproj(2)
                    scores(2)
                    av(2)
                    av(3)
                    kproj(3)
                    av(4)
                    av(5)
                    scores(3)
                    av(6)
                    av(7)

            # ================= phase C: O proj, LN2, MLP =================
            with tc.tile_pool(name="phc", bufs=1) as phc:
                pstatC_stack = ExitStack()
                pstatC = pstatC_stack.enter_context(
                    tc.tile_pool(name="pstatC", bufs=2, space="PSUM"))
                prepC = pstatC_stack.enter_context(
                    tc.tile_pool(name="prepC", bufs=2, space="PSUM"))
                pmmC_stack = ExitStack()
                pmmC = pmmC_stack.enter_context(
                    tc.tile_pool(name="pmmC", bufs=1, space="PSUM"))

                pm2 = pstatC.tile([1, 512], F32, name="pm2", tag="pm2")
                ps2 = pstatC.tile([1, 512], F32, name="ps2", tag="pm2")
                y2r = y2T
                pos = [pmmC.tile([P, 512], F32, name=f"po{m}", tag=f"po_{m}")
                       for m in range(KC)]
                # first half of O-proj contraction (heads 0-3) can start
                # as soon as RT8 pairs {0,1} exist
                for m in range(KC):
                    ms = slice(m * P, m * P + P)
                    nc.tensor.matmul(pos[m][:], Wo_s[:, 0:2, ms],
                                     RT8[:, 0:2, :],
                                     start=True, stop=False, perf_mode=DR)
                for m in range(KC):
                    ms = slice(m * P, m * P + P)
                    nc.tensor.matmul(pos[m][:], Wo_s[:, 2:4, ms],
                                     RT8[:, 2:4, :],
                                     start=False, stop=True, perf_mode=DR)
                    with nc.allow_low_precision(reason="f32r bits"):
                        nc.vector.scalar_tensor_tensor(
                            out=y2T[:, m, :], in0=pos[m][:], scalar=0.0,
                            in1=xo[:, m, :], op0=OP.add, op1=OP.add)
                    nc.tensor.matmul(pm2[:], onec_r[:], y2r[:, m, :],
                                     start=(m == 0), stop=(m == KC - 1))
                    sq2 = rtmp.tile([P, 512], BF16, name="sq2",
                                    tag=f"sq2_{m}")
                    with nc.allow_low_precision(reason="bf16 y2^2"):
                        nc.gpsimd.tensor_tensor(out=sq2[:], in0=y2T[:, m, :],
                                                in1=y2T[:, m, :], op=OP.mult)
                    nc.tensor.matmul(ps2[:], onec_b[:], sq2[:],
                                     start=(m == 0), stop=(m == KC - 1))
                pmmC_stack.close()
                S2_row = rows.tile([1, TC], BF16, name="S2_row")
                with nc.allow_low_precision(reason="bf16 rows"):
                    nc.vector.tensor_copy(out=S2_row[:], in_=pm2[:])
                t2_row = rtmp.tile([1, TC], F32, name="t2", tag="t_row")
                nc.scalar.activation(out=t2_row[:], in_=pm2[:],
                                     func=AF.Square)
                var2_row = rtmp.tile([1, TC], F32, name="var2",
                                     tag="var_row")
                nc.vector.scalar_tensor_tensor(
                    out=var2_row[:], in0=t2_row[:], scalar=-1.0 / 512.0,
                    in1=ps2[:], op0=OP.mult, op1=OP.add)
                ln2_row = rtmp.tile([1, TC], F32, name="ln2", tag="ln_row")
                nc.scalar.activation(out=ln2_row[:], in_=var2_row[:],
                                     func=AF.Ln, scale=1.0 / 512.0)
                rstd2_row = rows.tile([1, TC], BF16, name="rstd2_row")
                with nc.allow_low_precision(reason="bf16 rows"):
                    nc.scalar.activation(out=rstd2_row[:], in_=ln2_row[:],
                                         func=AF.Exp, scale=-0.5)
                # mean2 replicated (doesn't need rstd2 -> starts early);
                # a[m] = y2 - mean2 overlaps the ln/exp chain
                pm2rep = prepC.tile([P, 512], F32, name="pm2rep", tag="rep")
                nc.tensor.matmul(pm2rep[:], oned_row[:],
                                 S2_row[:],
                                 start=True, stop=True)
                pm2rep_sb = rtmp.tile([P, 512], BF16, name="m2rep_sb",
                                      tag="repsb")
                with nc.allow_low_precision(reason="bf16"):
                    nc.vector.tensor_copy(out=pm2rep_sb[:], in_=pm2rep[:])
                azs = []
                for m in range(KC):
                    a = rtmp.tile([P, TC], BF16, name="a_z",
                                  tag=f"az_{m}")
                    with nc.allow_low_precision(reason="bf16 z2"):
                        eng = nc.vector if m % 2 == 0 else nc.gpsimd
                        in1 = pm2rep[:] if m % 2 == 0 else pm2rep_sb[:]
                        eng.tensor_tensor(out=a[:], in0=y2T[:, m, :],
                                          in1=in1, op=OP.subtract)
                    azs.append(a)
                prrep2 = prepC.tile([P, 512], F32, name="prrep2", tag="rep")
                nc.tensor.matmul(prrep2[:], ones_row[:],
                                 rstd2_row[:],
                                 start=True, stop=True)
                prrep2_sb = rtmp.tile([P, 512], BF16, name="r2rep_sb",
                                      tag="repsb")
                with nc.allow_low_precision(reason="bf16"):
                    nc.vector.tensor_copy(out=prrep2_sb[:], in_=prrep2[:])
                z2b = phc.tile([P, KC, TC], BF16, name="z2b")
                for m in range(KC):
                    with nc.allow_low_precision(reason="bf16 z2"):
                        eng = nc.vector if m % 2 == 0 else nc.gpsimd
                        in1 = prrep2[:] if m % 2 == 0 else prrep2_sb[:]
                        eng.tensor_tensor(out=z2b[:, m, :], in0=azs[m][:],
                                          in1=in1, op=OP.mult)
                pstatC_stack.close()

                # MLP (bf16 for precision; fp8 here dominates the error)
                h1b = phc.tile([P, MC1, TC], BF16, name="h1b")
                ph2_stack = ExitStack()
                ph2 = ph2_stack.enter_context(
                    tc.tile_pool(name="ph2", bufs=1, space="PSUM"))
                p1p = ph2_stack.enter_context(
                    tc.tile_pool(name="p1p", bufs=2, space="PSUM"))
                p2s = [ph2.tile([P, 512], F32, name=f"p2_{m}", tag=f"p2_{m}")
                       for m in range(KC)]
                for k in range(MC1):
                    ks = slice(k * P, k * P + P)
                    p1 = p1p.tile([P, 512], F32, name="p1", tag="p1")
                    for kk in range(KC):
                        nc.tensor.matmul(p1[:], W1_s[:, kk, ks],
                                         z2b[:, kk, :],
                                         start=(kk == 0), stop=(kk == KC - 1))
                    with nc.allow_low_precision(reason="bf16 gelu"):
                        nc.scalar.activation(out=h1b[:, k, :], in_=p1[:],
                                             func=AF.Gelu,
                                             bias=ccol_sb[:, 4 + k:5 + k])
                    for m in range(KC):
                        ms = slice(m * P, m * P + P)
                        nc.tensor.matmul(
                            p2s[m][:], W2_s[:, k, ms], h1b[:, k, :],
                            start=(k == 0), stop=(k == MC1 - 1))

                out_sb = phc.tile([P, KC, TC], F32, name="out_sb")
                outT_r = outT.rearrange("(o p) t -> p o t", p=P)
                out_engs = [nc.sync, nc.gpsimd, nc.sync, nc.gpsimd]
                for m in range(KC):
                    nc.vector.scalar_tensor_tensor(
                        out=out_sb[:, m, :], in0=p2s[m][:],
                        scalar=ccol_sb[:, 12 + m:13 + m],
                        in1=y2T[:, m, :], op0=OP.add, op1=OP.add)
                    out_engs[m].dma_start(out=outT_r[:, m, :],
                                          in_=out_sb[:, m, :])
                ph2_stack.close()

    return _finalize(nc) if do_finalize else nc


def prep_inputs(y, Wq, bq, Wk, bk, Wv, bv, Wo, bo, ln1_g, ln1_b, ln2_g, ln2_b,
                W1, b1, W2, b2):
    """Host-side weight folding + fp8 quantization + per-core input maps."""
    f = np.float32
    f8 = ml_dtypes.float8_e4m3
    Wq_ = (Wq * ln1_g[:, None]).astype(f)
    Wk_ = (Wk * ln1_g[:, None]).astype(f)
    Wv_ = (Wv * ln1_g[:, None]).astype(f)
    bq_ = (ln1_b @ Wq + bq).astype(f)
    bv_ = (ln1_b @ Wv + bv).astype(f)
    bo_ = (bv_ @ Wo + bo).astype(f)
    W1_ = (W1 * ln2_g[:, None]).astype(f)
    b1_ = (ln2_b @ W1 + b1).astype(f)

    # crow rows are scaled x8 (matching the x8 weight quantization) and
    # by 1/512 where they multiply S = sum(x) rather than mean(x).
    crow = np.stack([
        8.0 * bq_,
        -8.0 * Wq_.sum(0) / 512.0,
        -8.0 * Wk_.sum(0) / 512.0,
        -8.0 * Wv_.sum(0) / 512.0,
    ]).astype(ml_dtypes.bfloat16)
    ccol = np.concatenate([
        np.zeros((P, 4), f), b1_.reshape(8, P).T,
        np.asarray(b2, f).reshape(4, P).T,
    ], axis=1).astype(f)

    shared = {
        "Wq8": np.ascontiguousarray((8.0 * Wq_).astype(f8)),
        "Wk8": np.ascontiguousarray((8.0 * Wk_).astype(f8)),
        "Wv8": np.ascontiguousarray((8.0 * Wv_).astype(f8)),
        "Wo8": np.ascontiguousarray(np.asarray(Wo, f).astype(f8)),
        "W18": np.ascontiguousarray(W1_.astype(ml_dtypes.bfloat16)),
        "W28": np.ascontiguousarray(np.asarray(W2, f).astype(
            ml_dtypes.bfloat16)),
        "crow": crow, "ccol": ccol,
    }
    in_maps = []
    for c in range(8):
        b, s = divmod(c, 4)
        ts = s * TC
        yTm = np.asarray(y, f)[b].T
        yrot = np.ascontiguousarray(np.roll(yTm, -ts, axis=1))
        in_maps.append({
            "y8T": yrot.astype(f8),
            "yoT": np.ascontiguousarray(yrot[:, 0:TC] + bo_[:, None]),
            **shared,
        })
    return in_maps


def gather_output(results):
    out = np.empty((B, N, D), np.float32)
    for c in range(8):
        b, s = divmod(c, 4)
        out[b, s * TC:(s + 1) * TC, :] = results[c]["outT"].T
    return out


_NC_CACHE = {}


def kernel(**inputs):
    """Full-input entry point: shard, run on 8 NeuronCores, gather."""
    from concourse.bass_utils import run_bass_kernel_spmd

    in_maps = prep_inputs(**{k: np.asarray(v) for k, v in inputs.items()})
    if "nc" not in _NC_CACHE:
        _NC_CACHE["nc"] = build_nc()
    nc = _NC_CACHE["nc"]
    res = run_bass_kernel_spmd(nc, in_maps, core_ids=list(range(8)))
    return gather_output(res.results)
